# revision 17
# baseline (speedup 1.0000x reference)
"""MultiHeadAttention (B=1, L=4096, D=768, H=12) on 8 trn2 NeuronCores.

Sharding: 2D — 4 head-groups (3 heads each) x 2 query-halves (2048 queries).
Each core projects K/V only for its 3 heads (4x less replicated projection
work than query-only sharding; collectives/RDMA are unusable here), runs
attention for its (heads x queries) block, and emits a PARTIAL output
projection [2048, 768] using its 192 rows of Wo. The host sums the 4
head-group partials per query half and adds the combined bias.

Precision: fp16 operands everywhere (fp8 matmul operands each cost >=1.1e-2
rel err vs the 2e-2 budget - measured). The two cheap tricks that survive
at fp16:
  - softmax exp is SPLIT between the Act engine (native Exp) and the DVE
    (Schraudolph in fp16: affine + saturating-rint to uint16 IS the fp16
    bit pattern of exp; noise ~0.05%). This halves the 164us exp wall.
  - an optional FP8_FRAC of kpos chunk-pairs runs the AV matmul in fp8
    DoubleRow (4x fewer PE cycles): exp output fp8 + vp stored fp8, both
    only for those chunks. Noise scales ~sqrt(FP8_FRAC)*1.7e-2.
Other structure: scores computed transposed [kpos, q] so softmax axis is
on partitions; denominator via a ones column in vp (AV row 64); per-pair
processing = one head x two 512-query blocks sharing exp instructions;
attention normalize on DVE+Pool (reciprocal / partition-broadcast / mult);
bk dropped (softmax-shift invariant), bv/bo folded into host-side bias.
"""

import numpy as np
import ml_dtypes

import concourse.bacc as bacc
import concourse.tile as tile
import concourse.mybir as mybir
from concourse.bass_utils import run_bass_kernel_spmd

P = 128
D_MODEL = 768
NUM_HEADS = 12
D_K = 64
NH_C = 3            # heads per core
DG = NH_C * D_K     # 192 projection dims per core
NE = 6              # input-dim tiles (contraction of projections)
NET = 2             # local output-dim tiles: 128 + 64
HA16 = 65           # fp16 vp row: 64 dims + ones col
HA8 = 66            # fp8 vp row: 64 dims + ones col + junk (DR needs even M)
SH = 2.0            # exp shift (cancels in softmax)
FP8_FRAC = 0.75      # fraction of kpos chunk-pairs using fp8 DoubleRow AV
ACT_FRAC = 0.54     # share of exp chunks on the Act engine
LN2 = float(np.log(2.0))

F32 = mybir.dt.float32
F16 = mybir.dt.float16
F8 = mybir.dt.float8e4
U8 = mybir.dt.uint8
U16 = mybir.dt.uint16
Act = mybir.ActivationFunctionType
Alu = mybir.AluOpType
DR = mybir.MatmulPerfMode.DoubleRow

# DVE Schraudolph constants: bits = rint(raw * SCALE + BIAS), saturating.
D16_SC = 0.125 * 1024.0 / LN2
D16_B = 15.0 * 1024.0 - SH * 1024.0 / LN2 - 58.9
D8_SC = 0.125 * 8.0 / LN2
D8_B = 56.0 - SH * 8.0 / LN2 - 0.46


def _sched(frac):
    acc = [0.0]

    def pick():
        acc[0] += frac
        if acc[0] >= 1.0 - 1e-9:
            acc[0] -= 1.0
            return True
        return False

    return pick


def build_program(L, n_cores):
    KT = L // P        # 32 kpos chunks
    LQC = L // 2       # queries per core (query half)
    NQB = LQC // 512   # 512-query blocks
    QCT = LQC // P     # output-projection query chunks
    HKT = KT // 2      # kpos chunks per vp half-tile

    nc = bacc.Bacc("TRN2", target_bir_lowering=False, debug=False,
                   num_devices=n_cores)

    qT = nc.dram_tensor("qT", [D_MODEL, LQC], F16, kind="ExternalInput").ap()
    kT = nc.dram_tensor("kT", [D_MODEL, L], F16, kind="ExternalInput").ap()
    vT = nc.dram_tensor("vT", [D_MODEL, L], F16, kind="ExternalInput").ap()
    WqT = nc.dram_tensor("WqT", [D_MODEL, DG], F16, kind="ExternalInput").ap()
    WkT = nc.dram_tensor("WkT", [D_MODEL, DG], F16, kind="ExternalInput").ap()
    WvT = nc.dram_tensor("WvT", [D_MODEL, DG], F16, kind="ExternalInput").ap()
    WoT = nc.dram_tensor("WoT", [2 * P, D_MODEL], F16, kind="ExternalInput").ap()
    bq_r = nc.dram_tensor("bq_r", [P, NET], F32, kind="ExternalInput").ap()
    out = nc.dram_tensor("out", [LQC, D_MODEL], F16, kind="ExternalOutput").ap()

    act_pick = _sched(0.65)      # Act share of quantize copies
    exp_act = _sched(ACT_FRAC)   # Act share of exp chunks
    fp8_pick = _sched(FP8_FRAC)  # fp8 share of AV chunk-pairs
    use16 = FP8_FRAC < 1.0 - 1e-9
    use8 = FP8_FRAC > 1e-9

    with tile.TileContext(nc) as tc:
        with (
            tc.tile_pool(name="persist", bufs=1) as persist,
            tc.tile_pool(name="dram", bufs=1, space="DRAM") as dram,
            tc.tile_pool(name="kt", bufs=2) as kt_pool,
            tc.tile_pool(name="vt", bufs=2) as vt_pool,
            tc.tile_pool(name="qt", bufs=2) as qt_pool,
            tc.tile_pool(name="stage", bufs=3) as stage,
            tc.tile_pool(name="vh", bufs=6) as vh_pool,
            tc.tile_pool(name="exp", bufs=3) as exp_pool,
            tc.tile_pool(name="small", bufs=2) as small,
            tc.tile_pool(name="outst", bufs=3) as outst,
            tc.tile_pool(name="psS", bufs=3, space="PSUM") as psS,  # 6 banks
            tc.tile_pool(name="psV", bufs=2, space="PSUM") as psV,  # 2 banks
        ):
            kpT = persist.tile([P, NET, L], F16)
            qpT = persist.tile([P, NET, LQC], F16)
            attnT = persist.tile([P, NET, LQC], F16)
            WqT_sb = persist.tile([P, NE, DG], F16)
            WkT_sb = persist.tile([P, NE, DG], F16)
            WvT_sb = persist.tile([P, NE, DG], F16)
            WoT_sb = persist.tile([P, NET, D_MODEL], F16)
            bq_sb = persist.tile([P, NET], F32)
            nsh_sb = persist.tile([P, 1], F32)
            nc.gpsimd.memset(nsh_sb[:], -SH)

            vp16_sb = persist.tile([P, KT, NH_C, HA16], F16,
                                   name="vp16_sb") if use16 else None
            vp8_sb = persist.tile([P, KT, NH_C, P], F8, name="vp8_sb") if use8 else None

            def load_wT(dst, src):
                nc.sync.dma_start(
                    out=dst[:], in_=src.rearrange("(t p) e -> p t e", p=P))

            load_wT(WqT_sb, WqT)
            nc.sync.dma_start(out=bq_sb[:], in_=bq_r)
            load_wT(WkT_sb, WkT)
            load_wT(WvT_sb, WvT)

            def qcopy(dst, src, bias=None):
                """PSUM->SBUF copy (opt. per-partition bias) on Act or DVE."""
                if act_pick():
                    if bias is None:
                        nc.scalar.activation(dst, src, Act.Identity)
                    else:
                        nc.scalar.activation(dst, src, Act.Identity, bias=bias)
                else:
                    if bias is None:
                        nc.vector.tensor_copy(out=dst, in_=src)
                    else:
                        nc.vector.tensor_scalar(out=dst, in0=src, scalar1=bias,
                                                scalar2=None, op0=Alu.add)

            # ---- Q projection: qpT[et, qb*512...] with bias ----
            for qb in range(NQB):
                qsl = slice(qb * 512, (qb + 1) * 512)
                qtin = qt_pool.tile([P, NE, 512], F16, tag="qt")
                nc.sync.dma_start(
                    out=qtin[:], in_=qT[:, qsl].rearrange("(t p) l -> p t l", p=P))
                ps = psS.tile([P, 1024], F32, name="sc")
                for d in range(NE):
                    nc.tensor.matmul(ps[:, 0:512], WqT_sb[:, d, 0:P],
                                     qtin[:, d, :], start=(d == 0),
                                     stop=(d == NE - 1))
                    nc.tensor.matmul(ps[0:64, 512:1024], WqT_sb[:, d, P:DG],
                                     qtin[:, d, :], start=(d == 0),
                                     stop=(d == NE - 1))
                qcopy(qpT[:, 0, qsl], ps[:, 0:512], bias=bq_sb[:, 0:1])
                qcopy(qpT[0:64, 1, qsl], ps[0:64, 512:1024],
                      bias=bq_sb[0:64, 1:2])

            # ---- K/V projections per 1024-wide l group ----
            for g in range(L // 1024):
                gsl = slice(g * 1024, (g + 1) * 1024)
                kt = kt_pool.tile([P, NE, 1024], F16, tag="kt")
                nc.sync.dma_start(
                    out=kt[:], in_=kT[:, gsl].rearrange("(t p) l -> p t l", p=P))
                vt = vt_pool.tile([P, NE, 1024], F16, tag="vt")
                nc.sync.dma_start(
                    out=vt[:], in_=vT[:, gsl].rearrange("(t p) l -> p t l", p=P))
                for et in range(NET):
                    esl = slice(et * P, min((et + 1) * P, DG))
                    np_ = P if et == 0 else 64
                    ps = psS.tile([P, 1024], F32, name="sc")
                    for half in range(2):
                        sl = slice(half * 512, half * 512 + 512)
                        for d in range(NE):
                            nc.tensor.matmul(ps[0:np_, sl], WkT_sb[:, d, esl],
                                             kt[:, d, sl], start=(d == 0),
                                             stop=(d == NE - 1))
                    qcopy(kpT[0:np_, et, gsl], ps[0:np_, :])
                for lt_loc in range(8):
                    lt = g * 8 + lt_loc
                    lsl = slice(lt_loc * P, (lt_loc + 1) * P)
                    ps = psS.tile([P, 1024], F32, name="sc")
                    for d in range(NE):
                        nc.tensor.matmul(ps[:, 0:DG], vt[:, d, lsl],
                                         WvT_sb[:, d, :], start=(d == 0),
                                         stop=(d == NE - 1))
                    src = ps[:, 0:DG].rearrange("p (h m) -> p h m", m=D_K)
                    if use16:
                        qcopy(vp16_sb[:, lt, :, 0:D_K], src)
                        nc.gpsimd.memset(vp16_sb[:, lt, :, D_K:HA16], 1.0)
                    if use8:
                        qcopy(vp8_sb[:, lt, :, 0:D_K], src)
                        nc.gpsimd.memset(vp8_sb[:, lt, :, D_K:HA8], 1.0)

            load_wT(WoT_sb, WoT)

            # ---- attention: 6 pairs = (3 heads) x (2 query-block pairs) ----
            def emit_oproj(qc):
                pso = psS.tile([P, 1024], F32, name="sc")
                qsl = slice(qc * P, (qc + 1) * P)
                for et, np_ in ((0, P), (1, 64)):
                    lhs = attnT[0:np_, et, qsl]
                    nc.tensor.matmul(pso[:, 0:512], lhs,
                                     WoT_sb[0:np_, et, 0:512],
                                     start=(et == 0), stop=(et == 1))
                    nc.tensor.matmul(pso[:, 512:768], lhs,
                                     WoT_sb[0:np_, et, 512:768],
                                     start=(et == 0), stop=(et == 1))
                ot = outst.tile([P, D_MODEL], F16, tag="ot")
                qcopy(ot[:], pso[:, 0:D_MODEL])
                nc.sync.dma_start(out=out[qc * P:(qc + 1) * P, :], in_=ot[:])

            for pair_i in range(NH_C * NQB // 2):
                qbp, hl = pair_i // NH_C, pair_i % NH_C
                if pair_i == NH_C:  # first query-half fully attended
                    for qc in range(QCT // 2):
                        emit_oproj(qc)
                et_h, pr = hl // 2, (hl % 2) * 64
                qbs = (2 * qbp, 2 * qbp + 1)
                avs_ps = [psV.tile([P, 512], F32, name="av") for _ in range(2)]
                def emit_av(ex, is8, cp):
                    first, last = cp == 0, cp == KT // 2 - 1
                    if is8:
                        for i in range(2):
                            nc.tensor.matmul(
                                avs_ps[i][:, :],
                                vp8_sb[:, 2 * cp:2 * cp + 2, hl, :],
                                ex[:, 0:2, i * 256:(i + 1) * 256].bitcast(F8),
                                start=first, stop=last, perf_mode=DR,
                                skip_group_check=True)
                    else:
                        for i in range(2):
                            for par in range(2):
                                nc.tensor.matmul(
                                    avs_ps[i][0:HA16, :],
                                    vp16_sb[:, 2 * cp + par, hl, 0:HA16],
                                    ex[:, par, i * 512:(i + 1) * 512],
                                    start=(first and par == 0),
                                    stop=(last and par == 1),
                                    skip_group_check=True)

                prev = None
                for cp in range(KT // 2):
                    is8 = fp8_pick()
                    ex = exp_pool.tile([P, 2, 1024], F16, tag="exp")
                    for par in range(2):
                        c = 2 * cp + par
                        ps_s = psS.tile([P, 1024], F32, name="sc")
                        for i in range(2):
                            nc.tensor.matmul(
                                ps_s[:, i * 512:(i + 1) * 512],
                                kpT[pr:pr + D_K, et_h, c * P:(c + 1) * P],
                                qpT[pr:pr + D_K, et_h,
                                    qbs[i] * 512:(qbs[i] + 1) * 512],
                                start=True, stop=True)
                        on_act = exp_act()
                        if is8:
                            dst = ex[:, par, 0:512]
                            if on_act:
                                nc.scalar.activation(dst.bitcast(F8), ps_s[:],
                                                     Act.Exp, scale=0.125,
                                                     bias=nsh_sb[:])
                            else:
                                nc.vector.tensor_scalar(
                                    out=dst.bitcast(U8), in0=ps_s[:],
                                    scalar1=D8_SC, scalar2=D8_B,
                                    op0=Alu.mult, op1=Alu.add)
                        else:
                            if on_act:
                                nc.scalar.activation(ex[:, par, :], ps_s[:],
                                                     Act.Exp, scale=0.125,
                                                     bias=nsh_sb[:])
                            else:
                                nc.vector.tensor_scalar(
                                    out=ex[:, par, :].bitcast(U16),
                                    in0=ps_s[:], scalar1=D16_SC, scalar2=D16_B,
                                    op0=Alu.mult, op1=Alu.add)
                    if prev is not None:
                        emit_av(*prev)
                    prev = (ex, is8, cp)
                emit_av(*prev)
                for i in range(2):
                    qsl = slice(qbs[i] * 512, (qbs[i] + 1) * 512)
                    av_s = small.tile([HA16, 512], F32, tag="avs")
                    qcopy(av_s[:], avs_ps[i][0:HA16, :])
                    recip = small.tile([1, 512], F32, tag="recip")
                    nc.vector.reciprocal(out=recip[:], in_=av_s[64:65, :])
                    rbc = small.tile([64, 512], F32, tag="rbc")
                    nc.gpsimd.partition_broadcast(rbc[:], recip[:])
                    nc.gpsimd.tensor_tensor(
                        out=attnT[pr:pr + D_K, et_h, qsl],
                        in0=av_s[0:D_K, :], in1=rbc[:], op=Alu.mult)

            # ---- remaining partial out (host adds bias + reduces) ----
            for qc in range(QCT // 2, QCT):
                emit_oproj(qc)

    nc.compile()
    return nc


def make_in_maps(q, k, v, Wq, bq, Wk, bk, Wv, bv, Wo, bo, L, LQ, n_cores):
    f32, f16 = np.float32, np.float16
    qT_full = np.ascontiguousarray(q[0].T, dtype=f16)       # [768, L]
    kT_full = np.ascontiguousarray(k[0].T, dtype=f16)
    vT_full = np.ascontiguousarray(v[0].T, dtype=f16)
    Wq, Wk, Wv, Wo = (np.asarray(x, f32) for x in (Wq, Wk, Wv, Wo))
    bq = np.asarray(bq, f32)
    LQC = L // 2
    maps = []
    for c in range(n_cores):
        hg, qh = c // 2, c % 2
        dsl = slice(hg * DG, (hg + 1) * DG)
        WoT_g = np.zeros((2 * P, D_MODEL), f16)
        WoT_g[0:DG] = Wo.T[dsl].astype(f16)
        bq_g = np.zeros((P, NET), f32)
        bq_g[:, 0] = bq[dsl][0:P]
        bq_g[0:64, 1] = bq[dsl][P:DG]
        maps.append({
            "qT": np.ascontiguousarray(qT_full[:, qh * LQC:(qh + 1) * LQC]),
            "kT": kT_full, "vT": vT_full,
            "WqT": np.ascontiguousarray(Wq.T[:, dsl].astype(f16)),
            "WkT": np.ascontiguousarray(Wk.T[:, dsl].astype(f16)),
            "WvT": np.ascontiguousarray(Wv.T[:, dsl].astype(f16)),
            "WoT": WoT_g,
            "bq_r": bq_g,
        })
    return maps


_PROGRAM_CACHE = {}


def get_program(L, LQ, n_cores):
    key = (L, n_cores)
    if key not in _PROGRAM_CACHE:
        _PROGRAM_CACHE[key] = build_program(L, n_cores)
    return _PROGRAM_CACHE[key]


def kernel(q, k, v, Wq, bq, Wk, bk, Wv, bv, Wo, bo):
    B, L, _ = q.shape
    assert B == 1
    n_cores = 8
    nc = get_program(L, L // 2, n_cores)
    in_maps = make_in_maps(q, k, v, Wq, bq, Wk, bk, Wv, bv, Wo, bo,
                           L, L // 2, n_cores)
    res = run_bass_kernel_spmd(nc, in_maps, core_ids=list(range(n_cores)))
    LQC = L // 2
    cb = (np.asarray(Wo, np.float32) @ np.asarray(bv, np.float32)
          + np.asarray(bo, np.float32))
    full = np.zeros((L, D_MODEL), np.float32)
    for c in range(n_cores):
        hg, qh = c // 2, c % 2
        full[qh * LQC:(qh + 1) * LQC] += res.results[c]["out"]
    full += cb
    return full[None].astype(np.float32)
